# revision 15
# baseline (speedup 1.0000x reference)
"""GameTheoreticAttention Trainium2 kernel.

Full inputs in, full output out. Internally: 8-way shard = 2 batches x 4
head-pairs. Core c handles batch n=c//4, heads {2j, 2j+1} (j=c%4), i.e. embed
columns [128j, 128j+128). Each core:
  - computes payoff softmax probs for q/k/v of its two heads on-device,
  - scales qT/kT by the q/k probs (free-axis broadcast via a tiny PE matmul),
  - builds PV stationary tiles = pv-scaled V blocks + a ones column (so the
    attention-softmax denominator Z falls out of the same matmul),
  - computes S^T = KW^T-tiles @ QW^T per (q-chunk, k-tile) in PSUM, exps it
    (ACT true-exp / DVE 1+x alternating; logits are ~1e-6 so both are exact
    to f32 rounding), accumulates O^T_unnorm and Z in PSUM,
  - normalizes O^T by 1/Z (GPSIMD row-broadcast + DVE reciprocal/mul),
  - applies its 128-row slice of w_out^T (row-parallel fc_out) and streams
    the partial [4096, 512] result to DRAM.
Host sums the 4 partials per batch and adds b_out.

All TensorEngine operands are bf16 (f32 matmul runs 2-pass LOW_HIGH at ~5x
the cost); accumulation stays f32 in PSUM. The payoff/normalization math
stays f32 on DVE/ACT.
"""

import os
import sys

for _p in ("/root/.axon_site", "/root/.axon_site/_ro/trn_rl_repo", "/opt/trn_rl_repo"):
    if os.path.isdir(_p) and _p not in sys.path:
        sys.path.append(_p)

import ml_dtypes
import numpy as np

import concourse.bass as bass  # noqa: E402
import concourse.tile as tile  # noqa: E402
from concourse import bacc, bass_isa, mybir  # noqa: E402
from concourse.bass_utils import run_bass_kernel_spmd  # noqa: E402

F32 = mybir.dt.float32
BF16 = mybir.dt.bfloat16
X = mybir.AxisListType.X
MULT = mybir.AluOpType.mult
ADD = mybir.AluOpType.add
EXP = mybir.ActivationFunctionType.Exp
BF = ml_dtypes.bfloat16

EMBED = 512
HEADS = 8
HD = 64
N = 2
L = 4096
NCORES = 8
NCH = 8  # 512-wide q chunks
NKT = 32  # 128-tall k tiles
INV_SQRT_E = float(1.0 / np.sqrt(512.0))


def build_program():
    nc = bacc.Bacc("TRN2", target_bir_lowering=False, debug=False)

    qT_d = nc.dram_tensor("qT", [128, L], BF16, kind="ExternalInput").ap()
    kT_d = nc.dram_tensor("kT", [128, L], BF16, kind="ExternalInput").ap()
    v_d = nc.dram_tensor("v", [L, 128], F32, kind="ExternalInput").ap()
    wt_d = nc.dram_tensor("wt", [128, EMBED], BF16, kind="ExternalInput").ap()
    wpay_d = nc.dram_tensor("wpay", [128, 6], BF16, kind="ExternalInput").ap()
    wvbc_d = nc.dram_tensor("wvbc", [128, 128], F32, kind="ExternalInput").ap()
    obd_d = nc.dram_tensor("obd", [2, 128], BF16, kind="ExternalInput").ap()
    y_d = nc.dram_tensor("y", [L, EMBED], F32, kind="ExternalOutput").ap()

    with tile.TileContext(nc) as tc:
        with (
            tc.tile_pool(name="persist", bufs=1) as persist,
            tc.tile_pool(name="sv", bufs=2) as sv_pool,
            tc.tile_pool(name="pqb", bufs=2) as pqb_pool,
            tc.tile_pool(name="e", bufs=6) as e_pool,
            tc.tile_pool(name="oz", bufs=2) as oz_pool,
            tc.tile_pool(name="zi", bufs=2) as zi_pool,
            tc.tile_pool(name="zbs", bufs=2) as zbs_pool,
            tc.tile_pool(name="on", bufs=3) as on_pool,
            tc.tile_pool(name="ysb", bufs=3) as y_pool,
            tc.tile_pool(name="ps_s", bufs=4, space="PSUM") as ps_s_pool,
            tc.tile_pool(name="ps_o", bufs=4, space="PSUM") as ps_o_pool,
        ):
            def ptile(shape, tag, dt=F32):
                return persist.tile(shape, dt, tag=tag, name=tag)

            qT = ptile([128, L], "qT_sb", BF16)
            kT = ptile([128, L], "kT_sb", BF16)
            v_sb = ptile([128, NKT, 128], "v_sb")
            wt_sb = ptile([64, 2, EMBED], "wt_sb", BF16)
            wpay_sb = ptile([128, 6], "wpay_sb", BF16)
            wvbc_sb = ptile([128, 2, 64], "wvbc_sb")
            obd_sb = ptile([2, 128], "obd_sb", BF16)
            vw_all = ptile([128, 64, 65], "vw_all", BF16)
            es_q = ptile([2, L], "es_q", BF16)
            es_k = ptile([2, L], "es_k", BF16)
            zq = ptile([2, 1], "zq")
            zk = ptile([2, 1], "zk")
            ziq = ptile([2, 1], "ziq")
            zik = ptile([2, 1], "zik")
            zobq = ptile([2, 128], "zobq", BF16)
            zobk = ptile([2, 128], "zobk", BF16)
            sv_col = ptile([128, NKT, 2], "sv_col")
            ev_col = ptile([128, NKT, 2], "ev_col")
            evp = ptile([128, 2], "evp")
            zvs = ptile([128, 2], "zvs")
            zvi = ptile([128, 2], "zvi")
            pv_col = ptile([128, NKT, 2], "pv_col")

            # ---- loads
            nc.sync.dma_start(qT[:], qT_d[:])
            nc.sync.dma_start(kT[:], kT_d[:])
            nc.sync.dma_start(v_sb[:], v_d.rearrange("(t p) e -> p t e", p=128))
            nc.sync.dma_start(wt_sb[:], wt_d.rearrange("(h p) e -> p h e", h=2))
            nc.sync.dma_start(wpay_sb[:], wpay_d[:])
            nc.sync.dma_start(wvbc_sb[:], wvbc_d.rearrange("p (h d) -> p h d", h=2))
            nc.sync.dma_start(obd_sb[:], obd_d[:])

            # ---- payoff scores for q, k (row layout, via PE) -> softmax rows
            for ti, (src, es, z, zi_, zob) in enumerate(
                ((qT, es_q, zq, ziq, zobq), (kT, es_k, zk, zik, zobk))
            ):
                for jc in range(NCH):
                    ps_pay = ps_s_pool.tile(
                        [2, 512], F32, tag="ps_s", name=f"ps_pay{ti}_{jc}"
                    )
                    nc.tensor.matmul(
                        ps_pay[:],
                        wpay_sb[:, 2 * ti : 2 * ti + 2],
                        src[:, 512 * jc : 512 * (jc + 1)],
                        start=True,
                        stop=True,
                    )
                    nc.scalar.activation(
                        es[:, 512 * jc : 512 * (jc + 1)], ps_pay[:], EXP
                    )
                nc.vector.reduce_sum(z[:], es[:], axis=X)
                nc.vector.reciprocal_approx_fast(zi_[:], z[:])
                # zob[r, m] = obd[r, m] / Z[r]: folds the softmax denominator
                # into the broadcast matmul's stationary operand
                nc.vector.tensor_scalar_mul(zob[:], obd_sb[:], zi_[:])

            # ---- payoff scores for v (column layout, via DVE on natural V)
            svt = sv_pool.tile([128, NKT, 2, 64], F32, tag="svt", name="svt")
            nc.vector.tensor_tensor(
                svt[:],
                v_sb[:].rearrange("p t (h d) -> p t h d", h=2),
                wvbc_sb[:].unsqueeze(1).broadcast_to([128, NKT, 2, 64]),
                op=MULT,
            )
            nc.vector.reduce_sum(sv_col[:].unsqueeze(3), svt[:], axis=X)
            nc.scalar.activation(ev_col[:], sv_col[:], EXP)
            for h in range(2):
                nc.vector.reduce_sum(
                    evp[:, h : h + 1], ev_col[:, :, h], axis=X
                )
            nc.gpsimd.partition_all_reduce(
                zvs[:], evp[:], channels=128, reduce_op=bass_isa.ReduceOp.add
            )
            nc.vector.reciprocal_approx_fast(zvi[:], zvs[:])
            for h in range(2):
                nc.vector.tensor_scalar_mul(
                    pv_col[:, :, h], ev_col[:, :, h], zvi[:, h : h + 1]
                )

            # ---- PV stationary tiles: pv-scaled V blocks + ones column for Z
            nc.vector.memset(vw_all[:, :, 64:65], 1.0)
            for h in range(2):
                nc.vector.tensor_tensor(
                    vw_all[:, 32 * h : 32 * h + 32, 0:64],
                    v_sb[:, :, 64 * h : 64 * (h + 1)],
                    pv_col[:, :, h].unsqueeze(2).broadcast_to([128, NKT, 64]),
                    op=MULT,
                )

            # ---- scale qT, kT in place by payoff probs (broadcast via PE)
            for ti, (src, es, zob) in enumerate(
                ((qT, es_q, zobq), (kT, es_k, zobk))
            ):
                for jc in range(NCH):
                    pqb = ps_s_pool.tile(
                        [128, 512], F32, tag="ps_s", name=f"pqb{ti}_{jc}"
                    )
                    nc.tensor.matmul(
                        pqb[:],
                        zob[:],
                        es[:, 512 * jc : 512 * (jc + 1)],
                        start=True,
                        stop=True,
                    )
                    pqb_sb = pqb_pool.tile(
                        [128, 512], BF16, tag="pqb_sb", name=f"pqb_sb{ti}_{jc}"
                    )
                    nc.scalar.copy(pqb_sb[:], pqb[:])
                    nc.gpsimd.tensor_mul(
                        src[:, 512 * jc : 512 * (jc + 1)],
                        src[:, 512 * jc : 512 * (jc + 1)],
                        pqb_sb[:],
                    )

            # ---- main attention + fc_out
            # Loop: h -> jc-pair group -> k-tile. Within a k-tile the two
            # S-matmuls share one stationary (LDWEIGHTS hides); O-matmuls for
            # k-tile t-1 issue after the S-matmuls of tile t so the exp
            # engines' latency never stalls PE.
            GRP = 2
            NG = NCH // GRP

            def normalize(h, jc, ps_o):
                oz = oz_pool.tile([64, 512], F32, tag="oz", name=f"oz_{jc}_{h}")
                nc.scalar.copy(oz[:], ps_o[0:64, :])
                zrow = zi_pool.tile([1, 512], F32, tag="zrow", name=f"zrow_{jc}_{h}")
                nc.scalar.copy(zrow[:], ps_o[64:65, :])
                zi = zi_pool.tile([1, 512], F32, tag="zi", name=f"zi_{jc}_{h}")
                # approx recip needs a base-partition-0 input (custom-DVE op)
                nc.vector.reciprocal_approx_fast(zi[:], zrow[:])
                zbs = zbs_pool.tile([64, 512], F32, tag="zbs", name=f"zbs_{jc}_{h}")
                nc.gpsimd.partition_broadcast(zbs[:], zi[:], channels=64)
                on = on_pool.tile(
                    [64, 512], BF16, tag="on", name=f"on_{jc}_{h}", bufs=18
                )
                nc.vector.tensor_tensor(on[:], oz[:], zbs[:], op=MULT)
                return on

            def fc_out(jc, on_h0, on_h1):
                for qq in range(4):
                    ps_y = ps_s_pool.tile(
                        [128, 512], F32, tag="ps_s", name=f"ps_y_{jc}_{qq}"
                    )
                    for h, on in enumerate((on_h0, on_h1)):
                        nc.tensor.matmul(
                            ps_y[:],
                            on[:, 128 * qq : 128 * (qq + 1)],
                            wt_sb[:, h, :],
                            start=(h == 0),
                            stop=(h == 1),
                            skip_group_check=True,
                        )
                    y_sb = y_pool.tile(
                        [128, 512], F32, tag="y_sb", name=f"y_sb_{jc}_{qq}"
                    )
                    if qq % 2 == 0:
                        nc.scalar.copy(y_sb[:], ps_y[:])
                    else:
                        nc.vector.tensor_copy(y_sb[:], ps_y[:])
                    r0 = (4 * jc + qq) * 128
                    nc.sync.dma_start(y_d[r0 : r0 + 128, :], y_sb[:])

            on_all = {}
            fc_ready = []
            for h in range(2):
                for g in range(NG):
                    jcs = [GRP * g + i for i in range(GRP)]
                    ps_os = {
                        jc: ps_o_pool.tile(
                            [65, 512], F32, tag="ps_o", name=f"ps_o_{jc}_{h}"
                        )
                        for jc in jcs
                    }
                    e_tiles = {}
                    for t in range(NKT + 1):
                        if t < NKT:
                            for gi, jc in enumerate(jcs):
                                ps_s = ps_s_pool.tile(
                                    [128, 512],
                                    F32,
                                    tag="ps_s",
                                    name=f"ps_s_{jc}_{h}_{t}",
                                )
                                nc.tensor.matmul(
                                    ps_s[:],
                                    kT[
                                        64 * h : 64 * (h + 1),
                                        128 * t : 128 * (t + 1),
                                    ],
                                    qT[
                                        64 * h : 64 * (h + 1),
                                        512 * jc : 512 * (jc + 1),
                                    ],
                                    start=True,
                                    stop=True,
                                )
                                e_sb = e_pool.tile(
                                    [128, 512],
                                    BF16,
                                    tag="e",
                                    name=f"e_{jc}_{h}_{t}",
                                    bufs=8,
                                )
                                if (t + gi) % 2 == 0:
                                    nc.scalar.activation(
                                        e_sb[:], ps_s[:], EXP, scale=INV_SQRT_E
                                    )
                                else:
                                    # exp(x) == 1 + x to bf16 rounding, |x|~1e-6
                                    nc.vector.tensor_scalar(
                                        e_sb[:],
                                        ps_s[:],
                                        INV_SQRT_E,
                                        1.0,
                                        op0=MULT,
                                        op1=ADD,
                                    )
                                e_tiles[(t, jc)] = e_sb
                        if t >= 1:
                            tt = t - 1
                            for jc in jcs:
                                nc.tensor.matmul(
                                    ps_os[jc][:],
                                    vw_all[:, 32 * h + tt, :],
                                    e_tiles.pop((tt, jc))[:],
                                    start=(tt == 0),
                                    stop=(tt == NKT - 1),
                                    skip_group_check=True,
                                )
                    for jc in jcs:
                        on_all[(h, jc)] = normalize(h, jc, ps_os[jc])
                    if h == 1:
                        fc_ready.append(jcs)
                        if len(fc_ready) > 1:
                            for jc in fc_ready.pop(0):
                                fc_out(jc, on_all[(0, jc)], on_all[(1, jc)])
            for jcs in fc_ready:
                for jc in jcs:
                    fc_out(jc, on_all[(0, jc)], on_all[(1, jc)])

    nc.compile()
    return nc


_NC = None


def _get_nc():
    global _NC
    if _NC is None:
        _NC = build_program()
    return _NC


def make_in_maps(values, keys, query, w_vp, w_kp, w_qp, w_out):
    values = np.ascontiguousarray(values, np.float32)
    keys = np.ascontiguousarray(keys, np.float32)
    query = np.ascontiguousarray(query, np.float32)
    w_vp = np.asarray(w_vp, np.float32)
    w_kp = np.asarray(w_kp, np.float32)
    w_qp = np.asarray(w_qp, np.float32)
    w_out = np.asarray(w_out, np.float32)

    wpay = np.zeros((128, 6), np.float32)
    wpay[0:64, 0] = w_qp
    wpay[64:128, 1] = w_qp
    wpay[0:64, 2] = w_kp
    wpay[64:128, 3] = w_kp
    wpay[0:64, 4] = w_vp
    wpay[64:128, 5] = w_vp
    wpay = wpay.astype(BF)
    wvbc = np.tile(np.concatenate([w_vp, w_vp])[None, :], (128, 1)).astype(np.float32)
    obd = np.zeros((2, 128), np.float32)
    obd[0, 0:64] = 1.0
    obd[1, 64:128] = 1.0
    obd = obd.astype(BF)
    wt_full = np.ascontiguousarray(w_out.T)  # [e_in, e_out]

    in_maps = []
    for c in range(NCORES):
        n, j = divmod(c, 4)
        e0 = j * 128
        in_maps.append(
            {
                "qT": np.ascontiguousarray(query[n, :, e0 : e0 + 128].T).astype(BF),
                "kT": np.ascontiguousarray(keys[n, :, e0 : e0 + 128].T).astype(BF),
                "v": np.ascontiguousarray(values[n, :, e0 : e0 + 128]),
                "wt": np.ascontiguousarray(wt_full[e0 : e0 + 128, :]).astype(BF),
                "wpay": wpay,
                "wvbc": wvbc,
                "obd": obd,
            }
        )
    return in_maps


def assemble(results, b_out):
    out = np.zeros((N, L, EMBED), np.float32)
    for c in range(NCORES):
        out[c // 4] += results[c]["y"]
    out += np.asarray(b_out, np.float32)[None, None, :]
    return out


def kernel(values, keys, query, w_vp, w_kp, w_qp, w_out, b_out):
    nc = _get_nc()
    in_maps = make_in_maps(values, keys, query, w_vp, w_kp, w_qp, w_out)
    res = run_bass_kernel_spmd(nc, in_maps, core_ids=list(range(NCORES)))
    return assemble(res.results, b_out)


# revision 16
# speedup vs baseline: 1.1050x; 1.1050x over previous
"""GameTheoreticAttention Trainium2 kernel.

Full inputs in, full output out. Internally: 8-way shard = 2 batches x 4
head-pairs. Core c handles batch n=c//4, heads {2j, 2j+1} (j=c%4), i.e. embed
columns [128j, 128j+128). Each core:
  - computes payoff softmax probs for q/k/v of its two heads on-device,
  - scales qT/kT by the q/k probs (free-axis broadcast via a tiny PE matmul),
  - builds PV stationary tiles = pv-scaled V blocks + a ones column (so the
    attention-softmax denominator Z falls out of the same matmul),
  - computes S^T = KW^T-tiles @ QW^T per (q-chunk, k-tile) in PSUM, exps it
    (ACT true-exp / DVE 1+x alternating; logits are ~1e-6 so both are exact
    to f32 rounding), accumulates O^T_unnorm and Z in PSUM,
  - normalizes O^T by 1/Z (GPSIMD row-broadcast + DVE reciprocal/mul),
  - applies its 128-row slice of w_out^T (row-parallel fc_out) and streams
    the partial [4096, 512] result to DRAM.
Host sums the 4 partials per batch and adds b_out.

All TensorEngine operands are bf16 (f32 matmul runs 2-pass LOW_HIGH at ~5x
the cost); accumulation stays f32 in PSUM. The payoff/normalization math
stays f32 on DVE/ACT.
"""

import os
import sys

for _p in ("/root/.axon_site", "/root/.axon_site/_ro/trn_rl_repo", "/opt/trn_rl_repo"):
    if os.path.isdir(_p) and _p not in sys.path:
        sys.path.append(_p)

import ml_dtypes
import numpy as np

import concourse.bass as bass  # noqa: E402
import concourse.tile as tile  # noqa: E402
from concourse import bacc, bass_isa, mybir  # noqa: E402
from concourse.bass_utils import run_bass_kernel_spmd  # noqa: E402

F32 = mybir.dt.float32
BF16 = mybir.dt.bfloat16
X = mybir.AxisListType.X
MULT = mybir.AluOpType.mult
ADD = mybir.AluOpType.add
EXP = mybir.ActivationFunctionType.Exp
BF = ml_dtypes.bfloat16

EMBED = 512
HEADS = 8
HD = 64
N = 2
L = 4096
NCORES = 8
NCH = 8  # 512-wide q chunks
NKT = 32  # 128-tall k tiles
INV_SQRT_E = float(1.0 / np.sqrt(512.0))


def build_program():
    nc = bacc.Bacc("TRN2", target_bir_lowering=False, debug=False)

    qT_d = nc.dram_tensor("qT", [128, L], BF16, kind="ExternalInput").ap()
    kT_d = nc.dram_tensor("kT", [128, L], BF16, kind="ExternalInput").ap()
    v_d = nc.dram_tensor("v", [L, 128], F32, kind="ExternalInput").ap()
    wt_d = nc.dram_tensor("wt", [128, EMBED], BF16, kind="ExternalInput").ap()
    wpay_d = nc.dram_tensor("wpay", [128, 6], BF16, kind="ExternalInput").ap()
    wvbc_d = nc.dram_tensor("wvbc", [128, 128], F32, kind="ExternalInput").ap()
    obd_d = nc.dram_tensor("obd", [2, 128], BF16, kind="ExternalInput").ap()
    y_d = nc.dram_tensor("y", [L, EMBED], F32, kind="ExternalOutput").ap()

    with tile.TileContext(nc) as tc:
        with (
            tc.tile_pool(name="persist", bufs=1) as persist,
            tc.tile_pool(name="sv", bufs=2) as sv_pool,
            tc.tile_pool(name="pqb", bufs=2) as pqb_pool,
            tc.tile_pool(name="e", bufs=6) as e_pool,
            tc.tile_pool(name="oz", bufs=2) as oz_pool,
            tc.tile_pool(name="zi", bufs=2) as zi_pool,
            tc.tile_pool(name="zbs", bufs=2) as zbs_pool,
            tc.tile_pool(name="on", bufs=3) as on_pool,
            tc.tile_pool(name="ysb", bufs=3) as y_pool,
            tc.tile_pool(name="ps_s", bufs=4, space="PSUM") as ps_s_pool,
            tc.tile_pool(name="ps_o", bufs=2, space="PSUM") as ps_o_pool,
            tc.tile_pool(name="ps_y", bufs=2, space="PSUM") as ps_y_pool,
        ):
            def ptile(shape, tag, dt=F32):
                return persist.tile(shape, dt, tag=tag, name=tag)

            qT = ptile([128, L], "qT_sb", BF16)
            kT = ptile([128, L], "kT_sb", BF16)
            v_sb = ptile([128, NKT, 128], "v_sb")
            wt_sb = ptile([64, 2, EMBED], "wt_sb", BF16)
            wpay_sb = ptile([128, 6], "wpay_sb", BF16)
            wvbc_sb = ptile([128, 2, 64], "wvbc_sb")
            obd_sb = ptile([2, 128], "obd_sb", BF16)
            vw_all = ptile([128, 64, 65], "vw_all", BF16)
            es_q = ptile([2, L], "es_q", BF16)
            es_k = ptile([2, L], "es_k", BF16)
            zq = ptile([2, 1], "zq")
            zk = ptile([2, 1], "zk")
            ziq = ptile([2, 1], "ziq")
            zik = ptile([2, 1], "zik")
            zobq = ptile([2, 128], "zobq", BF16)
            zobk = ptile([2, 128], "zobk", BF16)
            sv_col = ptile([128, NKT, 2], "sv_col")
            ev_col = ptile([128, NKT, 2], "ev_col")
            evp = ptile([128, 2], "evp")
            zvs = ptile([128, 2], "zvs")
            zvi = ptile([128, 2], "zvi")
            pv_col = ptile([128, NKT, 2], "pv_col")

            # ---- loads
            nc.sync.dma_start(qT[:], qT_d[:])
            nc.sync.dma_start(kT[:], kT_d[:])
            nc.sync.dma_start(v_sb[:], v_d.rearrange("(t p) e -> p t e", p=128))
            nc.sync.dma_start(wt_sb[:], wt_d.rearrange("(h p) e -> p h e", h=2))
            nc.sync.dma_start(wpay_sb[:], wpay_d[:])
            nc.sync.dma_start(wvbc_sb[:], wvbc_d.rearrange("p (h d) -> p h d", h=2))
            nc.sync.dma_start(obd_sb[:], obd_d[:])

            # ---- payoff scores for q, k (row layout, via PE) -> softmax rows
            for ti, (src, es, z, zi_, zob) in enumerate(
                ((qT, es_q, zq, ziq, zobq), (kT, es_k, zk, zik, zobk))
            ):
                for jc in range(NCH):
                    ps_pay = ps_y_pool.tile(
                        [2, 512], F32, tag="ps_y", name=f"ps_pay{ti}_{jc}"
                    )
                    nc.tensor.matmul(
                        ps_pay[:],
                        wpay_sb[:, 2 * ti : 2 * ti + 2],
                        src[:, 512 * jc : 512 * (jc + 1)],
                        start=True,
                        stop=True,
                    )
                    nc.scalar.activation(
                        es[:, 512 * jc : 512 * (jc + 1)], ps_pay[:], EXP
                    )
                nc.vector.reduce_sum(z[:], es[:], axis=X)
                nc.vector.reciprocal_approx_fast(zi_[:], z[:])
                # zob[r, m] = obd[r, m] / Z[r]: folds the softmax denominator
                # into the broadcast matmul's stationary operand
                nc.vector.tensor_scalar_mul(zob[:], obd_sb[:], zi_[:])

            # ---- payoff scores for v (column layout, via DVE on natural V)
            svt = sv_pool.tile([128, NKT, 2, 64], F32, tag="svt", name="svt")
            nc.vector.tensor_tensor(
                svt[:],
                v_sb[:].rearrange("p t (h d) -> p t h d", h=2),
                wvbc_sb[:].unsqueeze(1).broadcast_to([128, NKT, 2, 64]),
                op=MULT,
            )
            nc.vector.reduce_sum(sv_col[:].unsqueeze(3), svt[:], axis=X)
            nc.scalar.activation(ev_col[:], sv_col[:], EXP)
            for h in range(2):
                nc.vector.reduce_sum(
                    evp[:, h : h + 1], ev_col[:, :, h], axis=X
                )
            nc.gpsimd.partition_all_reduce(
                zvs[:], evp[:], channels=128, reduce_op=bass_isa.ReduceOp.add
            )
            nc.vector.reciprocal_approx_fast(zvi[:], zvs[:])
            for h in range(2):
                nc.vector.tensor_scalar_mul(
                    pv_col[:, :, h], ev_col[:, :, h], zvi[:, h : h + 1]
                )

            # ---- PV stationary tiles: pv-scaled V blocks + ones column for Z
            nc.vector.memset(vw_all[:, :, 64:65], 1.0)
            for h in range(2):
                nc.vector.tensor_tensor(
                    vw_all[:, 32 * h : 32 * h + 32, 0:64],
                    v_sb[:, :, 64 * h : 64 * (h + 1)],
                    pv_col[:, :, h].unsqueeze(2).broadcast_to([128, NKT, 64]),
                    op=MULT,
                )

            # ---- scale qT, kT in place by payoff probs (broadcast via PE)
            for ti, (src, es, zob) in enumerate(
                ((qT, es_q, zobq), (kT, es_k, zobk))
            ):
                for jc in range(NCH):
                    pqb = ps_y_pool.tile(
                        [128, 512], F32, tag="ps_y", name=f"pqb{ti}_{jc}"
                    )
                    nc.tensor.matmul(
                        pqb[:],
                        zob[:],
                        es[:, 512 * jc : 512 * (jc + 1)],
                        start=True,
                        stop=True,
                    )
                    pqb_sb = pqb_pool.tile(
                        [128, 512], BF16, tag="pqb_sb", name=f"pqb_sb{ti}_{jc}"
                    )
                    nc.scalar.copy(pqb_sb[:], pqb[:])
                    nc.gpsimd.tensor_mul(
                        src[:, 512 * jc : 512 * (jc + 1)],
                        src[:, 512 * jc : 512 * (jc + 1)],
                        pqb_sb[:],
                    )

            # ---- main attention + fc_out
            # Loop: h -> jc-pair group -> k-tile. Within a k-tile the two
            # S-matmuls share one stationary (LDWEIGHTS hides); O-matmuls for
            # k-tile t-1 issue after the S-matmuls of tile t so the exp
            # engines' latency never stalls PE.
            GRP = 2
            NG = NCH // GRP

            def normalize(h, jc, ps_o):
                oz = oz_pool.tile([64, 512], F32, tag="oz", name=f"oz_{jc}_{h}")
                nc.scalar.copy(oz[:], ps_o[0:64, :])
                zrow = zi_pool.tile([1, 512], F32, tag="zrow", name=f"zrow_{jc}_{h}")
                nc.scalar.copy(zrow[:], ps_o[64:65, :])
                zi = zi_pool.tile([1, 512], F32, tag="zi", name=f"zi_{jc}_{h}")
                # approx recip needs a base-partition-0 input (custom-DVE op)
                nc.vector.reciprocal_approx_fast(zi[:], zrow[:])
                zbs = zbs_pool.tile([64, 512], F32, tag="zbs", name=f"zbs_{jc}_{h}")
                nc.gpsimd.partition_broadcast(zbs[:], zi[:], channels=64)
                on = on_pool.tile(
                    [64, 512], BF16, tag="on", name=f"on_{jc}_{h}", bufs=18
                )
                nc.vector.tensor_tensor(on[:], oz[:], zbs[:], op=MULT)
                return on

            def fc_out(jc, on_h0, on_h1):
                for qq in range(4):
                    ps_y = ps_y_pool.tile(
                        [128, 512], F32, tag="ps_y", name=f"ps_y_{jc}_{qq}"
                    )
                    for h, on in enumerate((on_h0, on_h1)):
                        nc.tensor.matmul(
                            ps_y[:],
                            on[:, 128 * qq : 128 * (qq + 1)],
                            wt_sb[:, h, :],
                            start=(h == 0),
                            stop=(h == 1),
                            skip_group_check=True,
                        )
                    y_sb = y_pool.tile(
                        [128, 512], F32, tag="y_sb", name=f"y_sb_{jc}_{qq}"
                    )
                    if qq % 2 == 0:
                        nc.scalar.copy(y_sb[:], ps_y[:])
                    else:
                        nc.vector.tensor_copy(y_sb[:], ps_y[:])
                    r0 = (4 * jc + qq) * 128
                    nc.sync.dma_start(y_d[r0 : r0 + 128, :], y_sb[:])

            on_all = {}
            fc_ready = []
            for h in range(2):
                for g in range(NG):
                    jcs = [GRP * g + i for i in range(GRP)]
                    ps_os = {
                        jc: ps_o_pool.tile(
                            [65, 512], F32, tag="ps_o", name=f"ps_o_{jc}_{h}"
                        )
                        for jc in jcs
                    }
                    e_tiles = {}
                    for t in range(NKT + 1):
                        if t < NKT:
                            for gi, jc in enumerate(jcs):
                                ps_s = ps_s_pool.tile(
                                    [128, 512],
                                    F32,
                                    tag="ps_s",
                                    name=f"ps_s_{jc}_{h}_{t}",
                                )
                                nc.tensor.matmul(
                                    ps_s[:],
                                    kT[
                                        64 * h : 64 * (h + 1),
                                        128 * t : 128 * (t + 1),
                                    ],
                                    qT[
                                        64 * h : 64 * (h + 1),
                                        512 * jc : 512 * (jc + 1),
                                    ],
                                    start=True,
                                    stop=True,
                                )
                                e_sb = e_pool.tile(
                                    [128, 512],
                                    BF16,
                                    tag="e",
                                    name=f"e_{jc}_{h}_{t}",
                                    bufs=8,
                                )
                                if (t + gi) % 2 == 0:
                                    nc.scalar.activation(
                                        e_sb[:], ps_s[:], EXP, scale=INV_SQRT_E
                                    )
                                else:
                                    # exp(x) == 1 + x to bf16 rounding, |x|~1e-6
                                    nc.vector.tensor_scalar(
                                        e_sb[:],
                                        ps_s[:],
                                        INV_SQRT_E,
                                        1.0,
                                        op0=MULT,
                                        op1=ADD,
                                    )
                                e_tiles[(t, jc)] = e_sb
                        if t >= 1:
                            tt = t - 1
                            for jc in jcs:
                                nc.tensor.matmul(
                                    ps_os[jc][:],
                                    vw_all[:, 32 * h + tt, :],
                                    e_tiles.pop((tt, jc))[:],
                                    start=(tt == 0),
                                    stop=(tt == NKT - 1),
                                    skip_group_check=True,
                                )
                    for jc in jcs:
                        on_all[(h, jc)] = normalize(h, jc, ps_os[jc])
                    if h == 1:
                        fc_ready.append(jcs)
                        if len(fc_ready) > 1:
                            for jc in fc_ready.pop(0):
                                fc_out(jc, on_all[(0, jc)], on_all[(1, jc)])
            for jcs in fc_ready:
                for jc in jcs:
                    fc_out(jc, on_all[(0, jc)], on_all[(1, jc)])

    nc.compile()
    return nc


_NC = None


def _get_nc():
    global _NC
    if _NC is None:
        _NC = build_program()
    return _NC


def make_in_maps(values, keys, query, w_vp, w_kp, w_qp, w_out):
    values = np.ascontiguousarray(values, np.float32)
    keys = np.ascontiguousarray(keys, np.float32)
    query = np.ascontiguousarray(query, np.float32)
    w_vp = np.asarray(w_vp, np.float32)
    w_kp = np.asarray(w_kp, np.float32)
    w_qp = np.asarray(w_qp, np.float32)
    w_out = np.asarray(w_out, np.float32)

    wpay = np.zeros((128, 6), np.float32)
    wpay[0:64, 0] = w_qp
    wpay[64:128, 1] = w_qp
    wpay[0:64, 2] = w_kp
    wpay[64:128, 3] = w_kp
    wpay[0:64, 4] = w_vp
    wpay[64:128, 5] = w_vp
    wpay = wpay.astype(BF)
    wvbc = np.tile(np.concatenate([w_vp, w_vp])[None, :], (128, 1)).astype(np.float32)
    obd = np.zeros((2, 128), np.float32)
    obd[0, 0:64] = 1.0
    obd[1, 64:128] = 1.0
    obd = obd.astype(BF)
    wt_full = np.ascontiguousarray(w_out.T)  # [e_in, e_out]

    in_maps = []
    for c in range(NCORES):
        n, j = divmod(c, 4)
        e0 = j * 128
        in_maps.append(
            {
                "qT": np.ascontiguousarray(query[n, :, e0 : e0 + 128].T).astype(BF),
                "kT": np.ascontiguousarray(keys[n, :, e0 : e0 + 128].T).astype(BF),
                "v": np.ascontiguousarray(values[n, :, e0 : e0 + 128]),
                "wt": np.ascontiguousarray(wt_full[e0 : e0 + 128, :]).astype(BF),
                "wpay": wpay,
                "wvbc": wvbc,
                "obd": obd,
            }
        )
    return in_maps


def assemble(results, b_out):
    out = np.zeros((N, L, EMBED), np.float32)
    for c in range(NCORES):
        out[c // 4] += results[c]["y"]
    out += np.asarray(b_out, np.float32)[None, None, :]
    return out


def kernel(values, keys, query, w_vp, w_kp, w_qp, w_out, b_out):
    nc = _get_nc()
    in_maps = make_in_maps(values, keys, query, w_vp, w_kp, w_qp, w_out)
    res = run_bass_kernel_spmd(nc, in_maps, core_ids=list(range(NCORES)))
    return assemble(res.results, b_out)


# revision 17
# speedup vs baseline: 1.8206x; 1.6476x over previous
"""GameTheoreticAttention Trainium2 kernel.

Full inputs in, full output out. Internally: 8-way shard = 2 batches x 4
head-pairs. Core c handles batch n=c//4, heads {2j, 2j+1} (j=c%4), i.e. embed
columns [128j, 128j+128). Each core:
  - computes payoff softmax probs for q/k/v of its two heads on-device,
  - scales qT/kT by the q/k probs (free-axis broadcast via a tiny PE matmul),
  - builds PV stationary tiles = pv-scaled V blocks + a ones column (so the
    attention-softmax denominator Z falls out of the same matmul),
  - computes S^T = KW^T-tiles @ QW^T per (q-chunk, k-tile) in PSUM, exps it
    (ACT true-exp / DVE 1+x alternating; logits are ~1e-6 so both are exact
    to f32 rounding), accumulates O^T_unnorm and Z in PSUM,
  - normalizes O^T by 1/Z (GPSIMD row-broadcast + DVE reciprocal/mul),
  - applies its 128-row slice of w_out^T (row-parallel fc_out) and streams
    the partial [4096, 512] result to DRAM.
Host sums the 4 partials per batch and adds b_out.

All TensorEngine operands are bf16 (f32 matmul runs 2-pass LOW_HIGH at ~5x
the cost); accumulation stays f32 in PSUM. The payoff/normalization math
stays f32 on DVE/ACT.
"""

import os
import sys

for _p in ("/root/.axon_site", "/root/.axon_site/_ro/trn_rl_repo", "/opt/trn_rl_repo"):
    if os.path.isdir(_p) and _p not in sys.path:
        sys.path.append(_p)

import ml_dtypes
import numpy as np

import concourse.bass as bass  # noqa: E402
import concourse.tile as tile  # noqa: E402
from concourse import bacc, bass_isa, mybir  # noqa: E402
from concourse.bass_utils import run_bass_kernel_spmd  # noqa: E402

F32 = mybir.dt.float32
BF16 = mybir.dt.bfloat16
X = mybir.AxisListType.X
MULT = mybir.AluOpType.mult
ADD = mybir.AluOpType.add
EXP = mybir.ActivationFunctionType.Exp
BF = ml_dtypes.bfloat16

EMBED = 512
HEADS = 8
HD = 64
N = 2
L = 4096
NCORES = 8
NCH = 8  # 512-wide q chunks
NKT = 32  # 128-tall k tiles
INV_SQRT_E = float(1.0 / np.sqrt(512.0))


def build_program():
    nc = bacc.Bacc("TRN2", target_bir_lowering=False, debug=False)

    qT_d = nc.dram_tensor("qT", [128, L], BF16, kind="ExternalInput").ap()
    kT_d = nc.dram_tensor("kT", [128, L], BF16, kind="ExternalInput").ap()
    v_d = nc.dram_tensor("v", [L, 128], F32, kind="ExternalInput").ap()
    wt_d = nc.dram_tensor("wt", [128, EMBED], BF16, kind="ExternalInput").ap()
    wpay_d = nc.dram_tensor("wpay", [128, 6], BF16, kind="ExternalInput").ap()
    wvbc_d = nc.dram_tensor("wvbc", [128, 128], F32, kind="ExternalInput").ap()
    obd_d = nc.dram_tensor("obd", [2, 128], BF16, kind="ExternalInput").ap()
    y_d = nc.dram_tensor("y", [L, EMBED], F32, kind="ExternalOutput").ap()

    with tile.TileContext(nc) as tc:
        with (
            tc.tile_pool(name="persist", bufs=1) as persist,
            tc.tile_pool(name="sv", bufs=2) as sv_pool,
            tc.tile_pool(name="pqb", bufs=2) as pqb_pool,
            tc.tile_pool(name="e", bufs=6) as e_pool,
            tc.tile_pool(name="oz", bufs=2) as oz_pool,
            tc.tile_pool(name="zi", bufs=2) as zi_pool,
            tc.tile_pool(name="zbs", bufs=2) as zbs_pool,
            tc.tile_pool(name="on", bufs=3) as on_pool,
            tc.tile_pool(name="ysb", bufs=3) as y_pool,
            tc.tile_pool(name="ps_s", bufs=4, space="PSUM") as ps_s_pool,
            tc.tile_pool(name="ps_o", bufs=2, space="PSUM") as ps_o_pool,
            tc.tile_pool(name="ps_y", bufs=2, space="PSUM") as ps_y_pool,
        ):
            def ptile(shape, tag, dt=F32):
                return persist.tile(shape, dt, tag=tag, name=tag)

            qT = ptile([128, L], "qT_sb", BF16)
            qwT0 = ptile([128, L], "qwT0", BF16)
            qwT1 = ptile([128, L], "qwT1", BF16)
            kT = ptile([128, L], "kT_sb", BF16)
            v_sb = ptile([128, NKT, 128], "v_sb")
            wt_sb = ptile([128, EMBED], "wt_sb", BF16)
            wpay_sb = ptile([128, 6], "wpay_sb", BF16)
            wvbc_sb = ptile([128, 2, 64], "wvbc_sb")
            obd_sb = ptile([2, 128], "obd_sb", BF16)
            vw_all = ptile([128, 64, 65], "vw_all", BF16)
            es_q = ptile([2, L], "es_q", BF16)
            es_k = ptile([2, L], "es_k", BF16)
            zq = ptile([2, 1], "zq")
            zk = ptile([2, 1], "zk")
            ziq = ptile([2, 1], "ziq")
            zik = ptile([2, 1], "zik")
            zobq = ptile([2, 128], "zobq", BF16)
            zobk = ptile([2, 128], "zobk", BF16)
            sv_col = ptile([128, NKT, 2], "sv_col")
            ev_col = ptile([128, NKT, 2], "ev_col")
            evp = ptile([128, 2], "evp")
            zvs = ptile([128, 2], "zvs")
            zvi = ptile([128, 2], "zvi")
            pv_col = ptile([128, NKT, 2], "pv_col")

            # ---- loads
            nc.sync.dma_start(qT[:], qT_d[:])
            nc.sync.dma_start(kT[:], kT_d[:])
            nc.sync.dma_start(v_sb[:], v_d.rearrange("(t p) e -> p t e", p=128))
            nc.sync.dma_start(wt_sb[:], wt_d[:])
            nc.sync.dma_start(wpay_sb[:], wpay_d[:])
            nc.sync.dma_start(wvbc_sb[:], wvbc_d.rearrange("p (h d) -> p h d", h=2))
            nc.sync.dma_start(obd_sb[:], obd_d[:])

            # ---- payoff scores for q, k (row layout, via PE) -> softmax rows
            for ti, (src, es, z, zi_, zob) in enumerate(
                ((qT, es_q, zq, ziq, zobq), (kT, es_k, zk, zik, zobk))
            ):
                for jc in range(NCH):
                    ps_pay = ps_y_pool.tile(
                        [2, 512], F32, tag="ps_y", name=f"ps_pay{ti}_{jc}"
                    )
                    nc.tensor.matmul(
                        ps_pay[:],
                        wpay_sb[:, 2 * ti : 2 * ti + 2],
                        src[:, 512 * jc : 512 * (jc + 1)],
                        start=True,
                        stop=True,
                    )
                    nc.scalar.activation(
                        es[:, 512 * jc : 512 * (jc + 1)], ps_pay[:], EXP
                    )
                nc.vector.reduce_sum(z[:], es[:], axis=X)
                nc.vector.reciprocal_approx_fast(zi_[:], z[:])
                # zob[r, m] = obd[r, m] / Z[r]: folds the softmax denominator
                # into the broadcast matmul's stationary operand
                nc.vector.tensor_scalar_mul(zob[:], obd_sb[:], zi_[:])

            # ---- payoff scores for v (column layout, via DVE on natural V)
            svt = sv_pool.tile([128, NKT, 2, 64], F32, tag="svt", name="svt")
            nc.vector.tensor_tensor(
                svt[:],
                v_sb[:].rearrange("p t (h d) -> p t h d", h=2),
                wvbc_sb[:].unsqueeze(1).broadcast_to([128, NKT, 2, 64]),
                op=MULT,
            )
            nc.vector.reduce_sum(sv_col[:].unsqueeze(3), svt[:], axis=X)
            nc.scalar.activation(ev_col[:], sv_col[:], EXP)
            for h in range(2):
                nc.vector.reduce_sum(
                    evp[:, h : h + 1], ev_col[:, :, h], axis=X
                )
            nc.gpsimd.partition_all_reduce(
                zvs[:], evp[:], channels=128, reduce_op=bass_isa.ReduceOp.add
            )
            nc.vector.reciprocal_approx_fast(zvi[:], zvs[:])
            for h in range(2):
                nc.vector.tensor_scalar_mul(
                    pv_col[:, :, h], ev_col[:, :, h], zvi[:, h : h + 1]
                )

            # ---- PV stationary tiles: pv-scaled V blocks + ones column for Z
            nc.vector.memset(vw_all[:, :, 64:65], 1.0)
            for h in range(2):
                nc.vector.tensor_tensor(
                    vw_all[:, 32 * h : 32 * h + 32, 0:64],
                    v_sb[:, :, 64 * h : 64 * (h + 1)],
                    pv_col[:, :, h].unsqueeze(2).broadcast_to([128, NKT, 64]),
                    op=MULT,
                )

            # ---- apply payoff probs: kT in place; q into zero-padded
            # per-head copies so the S-matmul contracts over K=128 (the HAM
            # clock gate never leaves 1.2 GHz for K=64 matmuls)
            nc.vector.memset(qwT0[64:128, :], 0.0)
            nc.vector.memset(qwT1[0:64, :], 0.0)
            for ti, (es, zob) in enumerate(((es_q, zobq), (es_k, zobk))):
                for jc in range(NCH):
                    cs = slice(512 * jc, 512 * (jc + 1))
                    pqb = ps_y_pool.tile(
                        [128, 512], F32, tag="ps_y", name=f"pqb{ti}_{jc}"
                    )
                    nc.tensor.matmul(
                        pqb[:], zob[:], es[:, cs], start=True, stop=True
                    )
                    pqb_sb = pqb_pool.tile(
                        [128, 512], BF16, tag="pqb_sb", name=f"pqb_sb{ti}_{jc}"
                    )
                    nc.scalar.copy(pqb_sb[:], pqb[:])
                    if ti == 1:
                        nc.gpsimd.tensor_mul(kT[:, cs], kT[:, cs], pqb_sb[:])
                    else:
                        nc.vector.tensor_tensor(
                            qwT0[0:64, cs], qT[0:64, cs], pqb_sb[0:64, :], op=MULT
                        )
                        nc.vector.tensor_tensor(
                            qwT1[64:128, cs],
                            qT[64:128, cs],
                            pqb_sb[64:128, :],
                            op=MULT,
                        )

            # ---- main attention + fc_out
            # Loop: h -> jc-pair group -> k-tile. Within a k-tile the two
            # S-matmuls share one stationary (LDWEIGHTS hides); O-matmuls for
            # k-tile t-1 issue after the S-matmuls of tile t so the exp
            # engines' latency never stalls PE.
            GRP = 2
            NG = NCH // GRP

            def normalize(h, jc, ps_o):
                oz = oz_pool.tile([64, 512], F32, tag="oz", name=f"oz_{jc}_{h}")
                nc.scalar.copy(oz[:], ps_o[0:64, :])
                zrow = zi_pool.tile([1, 512], F32, tag="zrow", name=f"zrow_{jc}_{h}")
                nc.scalar.copy(zrow[:], ps_o[64:65, :])
                zi = zi_pool.tile([1, 512], F32, tag="zi", name=f"zi_{jc}_{h}")
                # approx recip needs a base-partition-0 input (custom-DVE op)
                nc.vector.reciprocal_approx_fast(zi[:], zrow[:])
                zbs = zbs_pool.tile([64, 512], F32, tag="zbs", name=f"zbs_{jc}_{h}")
                nc.gpsimd.partition_broadcast(zbs[:], zi[:], channels=64)
                if h == 0:
                    on_pair[jc] = on_pool.tile(
                        [128, 512], BF16, tag="on", name=f"on_{jc}", bufs=8
                    )
                nc.vector.tensor_tensor(
                    on_pair[jc][64 * h : 64 * (h + 1), :], oz[:], zbs[:], op=MULT
                )
                return on_pair[jc]

            def fc_out(jc, on_h0, on_h1):
                assert on_h0 is on_h1
                for qq in range(4):
                    ps_y = ps_y_pool.tile(
                        [128, 512], F32, tag="ps_y", name=f"ps_y_{jc}_{qq}"
                    )
                    nc.tensor.matmul(
                        ps_y[:],
                        on_h0[:, 128 * qq : 128 * (qq + 1)],
                        wt_sb[:],
                        start=True,
                        stop=True,
                    )
                    y_sb = y_pool.tile(
                        [128, 512], F32, tag="y_sb", name=f"y_sb_{jc}_{qq}"
                    )
                    if qq % 2 == 0:
                        nc.scalar.copy(y_sb[:], ps_y[:])
                    else:
                        nc.vector.tensor_copy(y_sb[:], ps_y[:])
                    r0 = (4 * jc + qq) * 128
                    nc.sync.dma_start(y_d[r0 : r0 + 128, :], y_sb[:])

            on_all = {}
            fc_ready = []
            on_pair = {}
            for h in range(2):
                for g in range(NG):
                    jcs = [GRP * g + i for i in range(GRP)]
                    ps_os = {
                        jc: ps_o_pool.tile(
                            [65, 512], F32, tag="ps_o", name=f"ps_o_{jc}_{h}"
                        )
                        for jc in jcs
                    }
                    e_tiles = {}
                    for t in range(NKT + 1):
                        if t < NKT:
                            for gi, jc in enumerate(jcs):
                                ps_s = ps_s_pool.tile(
                                    [128, 512],
                                    F32,
                                    tag="ps_s",
                                    name=f"ps_s_{jc}_{h}_{t}",
                                )
                                nc.tensor.matmul(
                                    ps_s[:],
                                    kT[:, 128 * t : 128 * (t + 1)],
                                    (qwT0 if h == 0 else qwT1)[
                                        :, 512 * jc : 512 * (jc + 1)
                                    ],
                                    start=True,
                                    stop=True,
                                )
                                e_sb = e_pool.tile(
                                    [128, 512],
                                    BF16,
                                    tag="e",
                                    name=f"e_{jc}_{h}_{t}",
                                    bufs=8,
                                )
                                if (t + gi) % 2 == 0:
                                    nc.scalar.activation(
                                        e_sb[:], ps_s[:], EXP, scale=INV_SQRT_E
                                    )
                                else:
                                    # exp(x) == 1 + x to bf16 rounding, |x|~1e-6
                                    nc.vector.tensor_scalar(
                                        e_sb[:],
                                        ps_s[:],
                                        INV_SQRT_E,
                                        1.0,
                                        op0=MULT,
                                        op1=ADD,
                                    )
                                e_tiles[(t, jc)] = e_sb
                        if t >= 1:
                            tt = t - 1
                            for jc in jcs:
                                nc.tensor.matmul(
                                    ps_os[jc][:],
                                    vw_all[:, 32 * h + tt, :],
                                    e_tiles.pop((tt, jc))[:],
                                    start=(tt == 0),
                                    stop=(tt == NKT - 1),
                                    skip_group_check=True,
                                )
                    for jc in jcs:
                        on_all[(h, jc)] = normalize(h, jc, ps_os[jc])
                    if h == 1:
                        fc_ready.append(jcs)
                        if len(fc_ready) > 1:
                            for jc in fc_ready.pop(0):
                                fc_out(jc, on_all[(0, jc)], on_all[(1, jc)])
            for jcs in fc_ready:
                for jc in jcs:
                    fc_out(jc, on_all[(0, jc)], on_all[(1, jc)])

    nc.compile()
    return nc


_NC = None


def _get_nc():
    global _NC
    if _NC is None:
        _NC = build_program()
    return _NC


def make_in_maps(values, keys, query, w_vp, w_kp, w_qp, w_out):
    values = np.ascontiguousarray(values, np.float32)
    keys = np.ascontiguousarray(keys, np.float32)
    query = np.ascontiguousarray(query, np.float32)
    w_vp = np.asarray(w_vp, np.float32)
    w_kp = np.asarray(w_kp, np.float32)
    w_qp = np.asarray(w_qp, np.float32)
    w_out = np.asarray(w_out, np.float32)

    wpay = np.zeros((128, 6), np.float32)
    wpay[0:64, 0] = w_qp
    wpay[64:128, 1] = w_qp
    wpay[0:64, 2] = w_kp
    wpay[64:128, 3] = w_kp
    wpay[0:64, 4] = w_vp
    wpay[64:128, 5] = w_vp
    wpay = wpay.astype(BF)
    wvbc = np.tile(np.concatenate([w_vp, w_vp])[None, :], (128, 1)).astype(np.float32)
    obd = np.zeros((2, 128), np.float32)
    obd[0, 0:64] = 1.0
    obd[1, 64:128] = 1.0
    obd = obd.astype(BF)
    wt_full = np.ascontiguousarray(w_out.T)  # [e_in, e_out]

    in_maps = []
    for c in range(NCORES):
        n, j = divmod(c, 4)
        e0 = j * 128
        in_maps.append(
            {
                "qT": np.ascontiguousarray(query[n, :, e0 : e0 + 128].T).astype(BF),
                "kT": np.ascontiguousarray(keys[n, :, e0 : e0 + 128].T).astype(BF),
                "v": np.ascontiguousarray(values[n, :, e0 : e0 + 128]),
                "wt": np.ascontiguousarray(wt_full[e0 : e0 + 128, :]).astype(BF),
                "wpay": wpay,
                "wvbc": wvbc,
                "obd": obd,
            }
        )
    return in_maps


def assemble(results, b_out):
    out = np.zeros((N, L, EMBED), np.float32)
    for c in range(NCORES):
        out[c // 4] += results[c]["y"]
    out += np.asarray(b_out, np.float32)[None, None, :]
    return out


def kernel(values, keys, query, w_vp, w_kp, w_qp, w_out, b_out):
    nc = _get_nc()
    in_maps = make_in_maps(values, keys, query, w_vp, w_kp, w_qp, w_out)
    res = run_bass_kernel_spmd(nc, in_maps, core_ids=list(range(NCORES)))
    return assemble(res.results, b_out)
